# revision 50
# baseline (speedup 1.0000x reference)
"""BiAttention kernel for Trainium2, 8-core data-parallel SPMD.

Computes (per batch):
  x1p = relu(x1 @ W1.T + b1);  x2p = relu(x2 @ W2.T + b2)
  sim = x1p @ x2p.T  (masked with x2_mask cols / x1_mask rows)
  attn_a = rowsoftmax(sim | x2mask) @ x2
  attn_b = colsoftmax(sim | both masks).T @ x1   (all-NEG columns -> uniform mean)

Strategy: shard batch (16) across 8 cores (2 each). fp32r (TF32-rate) matmuls
for the projection/sim chain. The softmax-weight matmuls (attn_a / attn_b) run
as fp8e4 DoubleRow matmuls (2 K-planes per instruction, 0.5 cyc/row): weights
are per-row rescaled to a ~224 peak (the scale cancels in the softmax
normalization; 224 uses fp8's full 17-octave range so deep-tail entries
survive) and split into fp8 hi + fp8 lo residual; values are host-split into
fp8 hi+lo. Each output psum accumulates three products hi*vh + hi*vl + lo*vh
(lo*vl dropped, ~4e-4). Row/col maxes come from free-dim DVE max-reduces of
the G tiles / transposed F strips; 1/max is broadcast along the free dim via
rank-1 ones matmuls. Softmax sums via fp8-ones DoubleRow matmuls on the
quantized weights, so numerator and denominator stay consistent. The x1p
projection is computed per n-half (interleaved with phase A) to fit SBUF.
x1_mask handled by host-zeroing x1 value rows + a keep column for the
col-softmax denominator; fully-masked columns blend to the uniform mean via
an indicator K=1 matmul adding [colsum_x1 | 2048] before the division.
"""
import os
import sys

sys.path.insert(0, "/opt/trn_rl_repo")

_NO_DR = bool(os.environ.get("K_NO_DR"))  # debug: regular fp8 matmuls
_NO_TTR = bool(os.environ.get("K_NO_TTR"))  # debug: no tensor_tensor_reduce
_NO_ACT8 = bool(os.environ.get("K_NO_ACT8"))  # debug: DVE fp8 copy

import numpy as np
import ml_dtypes

import concourse.bass as bass  # noqa: F401
import concourse.bacc as bacc
import concourse.tile as tile
from concourse import mybir
from concourse.bass_utils import run_bass_kernel_spmd

# ---- problem constants (hardcoded per harness contract) ----
B, Nn, Mm, D = 16, 2048, 2048, 1024
NCORES = 8
BPC = B // NCORES
P = 128
ET, DT, NT, MT = D // P, D // P, Nn // P, Mm // P
NEG = -2e20
C_SHIFT = 75.0
QTOP = 224.0  # fp8 weight-scale target peak

F32 = mybir.dt.float32
F32R = mybir.dt.float32r
BF16 = mybir.dt.bfloat16
FP8 = mybir.dt.float8e4
BF16_NP = ml_dtypes.bfloat16
FP8_NP = ml_dtypes.float8_e4m3

Relu = mybir.ActivationFunctionType.Relu
Exp = mybir.ActivationFunctionType.Exp
Copy = mybir.ActivationFunctionType.Copy
Mult = mybir.AluOpType.mult
Max = mybir.AluOpType.max
Add = mybir.AluOpType.add
Subtract = mybir.AluOpType.subtract
AxX = mybir.AxisListType.X
DR = mybir.MatmulPerfMode.DoubleRow


def _emit(nc):
    dram = nc.dram_tensor
    x1t = dram("x1t", [BPC, DT, P, Nn], F32, kind="ExternalInput")  # x1.T  [d, n]
    x2t = dram("x2t", [BPC, DT, P, Mm], F32, kind="ExternalInput")
    w1t = dram("w1t", [DT, P, D], F32, kind="ExternalInput")  # W1.T [d, e]
    w2t = dram("w2t", [DT, P, D], F32, kind="ExternalInput")
    b1c = dram("b1c", [P, ET], F32, kind="ExternalInput")
    b2c = dram("b2c", [P, ET], F32, kind="ExternalInput")
    x1h8 = dram("x1h8", [BPC, NT, P, D], FP8, kind="ExternalInput")  # masked rows 0
    x1l8 = dram("x1l8", [BPC, NT, P, D], FP8, kind="ExternalInput")
    x2h8 = dram("x2h8", [BPC, MT, P, D], FP8, kind="ExternalInput")
    x2l8 = dram("x2l8", [BPC, MT, P, D], FP8, kind="ExternalInput")
    x2mbc = dram("x2mbc", [BPC, P, MT], F32, kind="ExternalInput")  # NEG*m2 - C
    m2k = dram("m2k", [BPC, P, MT], F32, kind="ExternalInput")  # 2048*m2 col
    keep8s = dram("keep8s", [BPC, P, NT, P], FP8, kind="ExternalInput")  # bcast cols
    m2i = dram("m2i", [BPC, 1, Mm], BF16, kind="ExternalInput")  # m2 as 0/1 row
    blr = dram("blr", [BPC, 1, D], BF16, kind="ExternalInput")  # colsum_x1 row
    keeprow = dram("keeprow", [BPC, 1, Nn], BF16, kind="ExternalInput")  # ~x1_mask
    keepstrip = dram("keepstrip", [BPC, P, NT, P], FP8, kind="ExternalInput")
    ident = dram("ident", [P, P], BF16, kind="ExternalInput")  # transpose identity
    ones8 = dram("ones8", [P, 2, P], FP8, kind="ExternalInput")  # DR ones lhsT
    onesrow = dram("onesrow", [1, P], BF16, kind="ExternalInput")  # rank-1 lhsT
    one11 = dram("one11", [1, 1], F32, kind="ExternalInput")
    c2048 = dram("c2048", [1, 1], BF16, kind="ExternalInput")
    outa = dram("outa", [BPC, NT, P, D], F32, kind="ExternalOutput")
    outb = dram("outb", [BPC, MT, P, D], F32, kind="ExternalOutput")

    with tile.TileContext(nc) as tc:
        import contextlib

        with contextlib.ExitStack() as ctx:
            pp = ctx.enter_context(tc.tile_pool(name="persist", bufs=1))
            x2l = ctx.enter_context(tc.tile_pool(name="x2lhs", bufs=2))
            big = ctx.enter_context(tc.tile_pool(name="big32", bufs=1))
            q8p = ctx.enter_context(tc.tile_pool(name="quant8", bufs=1))
            f8b = ctx.enter_context(tc.tile_pool(name="fp8b", bufs=2))
            prs = ctx.enter_context(tc.tile_pool(name="projrhs", bufs=2))
            vst = ctx.enter_context(tc.tile_pool(name="vals", bufs=2))
            fts = ctx.enter_context(tc.tile_pool(name="ftstrip", bufs=2))
            stg = ctx.enter_context(tc.tile_pool(name="stage", bufs=2))
            tst = ctx.enter_context(tc.tile_pool(name="tstage", bufs=2))
            tmpq = ctx.enter_context(tc.tile_pool(name="tmpq", bufs=2))
            rows = ctx.enter_context(tc.tile_pool(name="rows", bufs=1))
            scr = ctx.enter_context(tc.tile_pool(name="scrow", bufs=1))
            sml = ctx.enter_context(tc.tile_pool(name="small", bufs=2))
            cst = ctx.enter_context(tc.tile_pool(name="consts", bufs=1))
            dsc = ctx.enter_context(tc.tile_pool(name="dramscr", bufs=2, space="DRAM"))
            psum = ctx.enter_context(tc.tile_pool(name="psum", bufs=8, space="PSUM"))

            # constants
            b1c_t = cst.tile([P, ET], F32, tag="b1c")
            b2c_t = cst.tile([P, ET], F32, tag="b2c")
            ident_t = cst.tile([P, P], BF16, tag="ident")
            ones8_t = cst.tile([P, 2, P], FP8, tag="ones8")
            onesrow_t = cst.tile([1, P], BF16, tag="onesrow")
            one11_t = cst.tile([1, 1], F32, tag="one11")
            c2048_t = cst.tile([1, 1], BF16, tag="c2048")
            nc.sync.dma_start(out=b1c_t, in_=b1c.ap())
            nc.sync.dma_start(out=b2c_t, in_=b2c.ap())
            nc.sync.dma_start(out=ident_t, in_=ident.ap())
            nc.sync.dma_start(out=ones8_t, in_=ones8.ap())
            nc.sync.dma_start(out=onesrow_t, in_=onesrow.ap())
            nc.sync.dma_start(out=one11_t, in_=one11.ap())
            nc.sync.dma_start(out=c2048_t, in_=c2048.ap())

            NCH = 256

            def dr_matmul(out, lhsT, rhs, start, stop):
                """DoubleRow matmul with an env-togglable plain-fp8 fallback.

                lhsT [K,2,M], rhs [K,2,N2] with N2=2*out free; out [M, N]."""
                if not _NO_DR:
                    nc.tensor.matmul(
                        out, lhsT, rhs, start=start, stop=stop, perf_mode=DR
                    )
                    return
                for i in range(2):
                    nc.tensor.matmul(
                        out,
                        lhsT[:, i, :],
                        rhs[:, i, :],
                        start=(start and i == 0),
                        stop=(stop and i == 1),
                    )

            def proj_chunk(writer, xt, wt, w_t, bc, bb, nch):
                """Project one NCH-column chunk; writer(et, ps_ap) consumes."""
                rhs_t = prs.tile([P, DT, NCH], F32R, tag="prhs")
                for dt_ in range(0, DT, 2):
                    nc.sync.dma_start(
                        out=rhs_t[:, dt_ : dt_ + 2, :],
                        in_=xt.ap()[
                            bb : bb + 1, dt_ : dt_ + 2, :, nch * NCH : (nch + 1) * NCH
                        ]
                        .rearrange("o dt p n -> p (o dt) n")
                        .bitcast(F32R),
                    )
                for et in range(ET):
                    ps = psum.tile([P, 512], F32, tag="ps")
                    for dt_ in range(DT):
                        nc.tensor.matmul(
                            ps[:, :NCH],
                            w_t[:, dt_, et * P : (et + 1) * P],
                            rhs_t[:, dt_, :],
                            start=(dt_ == 0),
                            stop=(dt_ == DT - 1),
                        )
                    writer(et, ps[:, :NCH], bc)

            for b in range(BPC):
                x2mbc_t = sml.tile([P, MT], F32, tag="x2mbc")
                m2k_t = sml.tile([P, MT], F32, tag="m2k")
                keep8_t = rows.tile([P, NT, P], FP8, tag="keep8")
                nc.sync.dma_start(
                    out=x2mbc_t, in_=x2mbc.ap()[b : b + 1].rearrange("o p t -> p (o t)")
                )
                nc.sync.dma_start(
                    out=m2k_t, in_=m2k.ap()[b : b + 1].rearrange("o p t -> p (o t)")
                )
                nc.sync.dma_start(
                    out=keep8_t,
                    in_=keep8s.ap()[b : b + 1].rearrange("o p t m -> p (o t) m"),
                )
                m2i_t = rows.tile([1, Mm], BF16, tag="m2i")
                blr_t = rows.tile([1, D], BF16, tag="blr")
                keepr_t = rows.tile([1, Nn], BF16, tag="keepr")
                nc.sync.dma_start(
                    out=keepr_t,
                    in_=keeprow.ap()[b : b + 1].rearrange("o r n -> (o r) n"),
                )
                # keep[n] broadcast along m, strip layout (zeroes x1-masked
                # rows of F so they can't overflow the kept-colmax fp8 scale)
                kstrip_t = rows.tile([P, NT, P], FP8, tag="kstrip")
                nc.sync.dma_start(
                    out=kstrip_t,
                    in_=keepstrip.ap()[b : b + 1].rearrange("o p t m -> p (o t) m"),
                )
                nc.sync.dma_start(
                    out=m2i_t, in_=m2i.ap()[b : b + 1].rearrange("o r m -> (o r) m")
                )
                nc.sync.dma_start(
                    out=blr_t, in_=blr.ap()[b : b + 1].rearrange("o r m -> (o r) m")
                )
                srow_rec = sml.tile([P, NT], F32, tag="srr")
                scol_rec = sml.tile([P, MT], F32, tag="scr")
                cmax_h = sml.tile([P, 2, MT], F32, tag="cmaxh")  # per-half col maxes
                fscr = dsc.tile([NT, P, Mm], BF16, tag="fscr")  # F[n, m] scratch

                # ---- PHASE P: x2 projection ([e, m] layout, f32r) ----
                # spilled to DRAM scratch; sim reloads per-m-tile lhsT slices
                x2pd = dsc.tile([ET, P, Mm], F32R, tag="x2pd")
                w_t = big.tile([P, DT, D], F32R, tag="big")
                for dt_ in range(DT):
                    nc.sync.dma_start(
                        out=w_t[:, dt_, :],
                        in_=w2t.ap()[dt_ : dt_ + 1]
                        .rearrange("t p e -> p (t e)")
                        .bitcast(F32R),
                    )

                for nch in range(Mm // NCH):

                    def w2_writer(et, ps_ap, bc, _off=nch * NCH):
                        pstage = stg.tile([P, NCH], F32R, tag="pstage")
                        nc.scalar.activation(
                            pstage, ps_ap, Relu, bias=bc[:, et : et + 1], scale=1.0
                        )
                        nc.sync.dma_start(
                            out=x2pd[et : et + 1, :, _off : _off + NCH].rearrange(
                                "e p m -> p (e m)"
                            ),
                            in_=pstage,
                        )

                    proj_chunk(w2_writer, x2t, w2t, w_t, b2c_t, b, nch)

                # ---- PHASE A (per n-half): x1 proj; simT -> G; quantize;
                #      s_row; attn_a
                for h in range(2):
                    # x1 projection for this n-half only (SBUF economy)
                    x1p = pp.tile([P, ET, 1024], F32R, tag="x1p")
                    w_t = big.tile([P, DT, D], F32R, tag="big")
                    for dt_ in range(DT):
                        nc.sync.dma_start(
                            out=w_t[:, dt_, :],
                            in_=w1t.ap()[dt_ : dt_ + 1]
                            .rearrange("t p e -> p (t e)")
                            .bitcast(F32R),
                        )
                    for c in range(1024 // NCH):

                        def w1_writer(et, ps_ap, bc, _off=c * NCH):
                            nc.scalar.activation(
                                x1p[:, et, _off : _off + NCH],
                                ps_ap,
                                Relu,
                                bias=bc[:, et : et + 1],
                                scale=1.0,
                            )

                        proj_chunk(w1_writer, x1t, w1t, w_t, b1c_t, b, h * 4 + c)

                    # keep-row broadcast for kept-colmax (per-free-n mask)
                    kbc_h = rows.tile([P, 1024], BF16, tag="kbch")
                    for c2 in range(2):
                        ps_kb = psum.tile([P, 512], F32, tag="ps")
                        nc.tensor.matmul(
                            ps_kb,
                            onesrow_t,
                            keepr_t[0:1, h * 1024 + c2 * 512 : h * 1024 + (c2 + 1) * 512],
                            start=True,
                            stop=True,
                        )
                        nc.vector.tensor_copy(kbc_h[:, c2 * 512 : (c2 + 1) * 512], ps_kb)

                    g_t = big.tile([P, MT, 1024], BF16, tag="big")
                    for mt in range(MT):
                        x2l_t = x2l.tile([P, ET, P], F32R, tag="x2l")
                        nc.sync.dma_start(
                            out=x2l_t,
                            in_=x2pd[:, :, mt * P : (mt + 1) * P].rearrange(
                                "e p m -> p e m"
                            ),
                        )
                        for c2 in range(2):
                            nlo = c2 * 512
                            ps = psum.tile([P, 512], F32, tag="ps")
                            for et in range(ET):
                                nc.tensor.matmul(
                                    ps,
                                    x2l_t[:, et, :],
                                    x1p[:, et, nlo : nlo + 512],
                                    start=(et == 0),
                                    stop=(et == ET - 1),
                                )
                            nc.scalar.activation(
                                g_t[:, mt, c2 * 512 : (c2 + 1) * 512],
                                ps,
                                Exp,
                                bias=x2mbc_t[:, mt : mt + 1],
                                scale=1.0,
                            )
                        # kept-colmax partial (over this n-half) for this m-tile:
                        # max over keep-masked n so the fp8 scale matches the
                        # reference col-softmax's effective max
                        gkq = tmpq.tile([P, 1024], BF16, tag="tq")
                        if _NO_TTR:
                            nc.vector.tensor_tensor(
                                out=gkq, in0=g_t[:, mt, :], in1=kbc_h, op=Mult
                            )
                            nc.vector.tensor_reduce(
                                cmax_h[:, h, mt : mt + 1], gkq, AxX, Max
                            )
                        else:
                            nc.vector.tensor_tensor_reduce(
                                out=gkq,
                                in0=g_t[:, mt, :],
                                in1=kbc_h,
                                scale=1.0,
                                scalar=0.0,
                                op0=Mult,
                                op1=Max,
                                accum_out=cmax_h[:, h, mt : mt + 1],
                            )
                    # transposes: F[n, m] blocks -> DRAM scratch; rowmax accum
                    rmax_h = sml.tile([P, 8], F32, tag="rmaxh")
                    for mt in range(MT):
                        tst_b = tst.tile([P, 8, P], BF16, tag="tst")
                        ps_t8 = psum.tile([P, 8, P], BF16, tag="ps")
                        for ntl in range(8):
                            nc.tensor.transpose(
                                ps_t8[:, ntl, :],
                                g_t[:, mt, ntl * P : (ntl + 1) * P],
                                ident_t,
                            )
                        # rowmax (attn_a scale) from the UNMASKED transpose
                        if mt == 0:
                            nc.vector.tensor_reduce(rmax_h, ps_t8, AxX, Max)
                        else:
                            rtmp = sml.tile([P, 8], F32, tag="rtmp")
                            nc.vector.tensor_reduce(rtmp, ps_t8, AxX, Max)
                            nc.vector.tensor_tensor(
                                out=rmax_h, in0=rmax_h, in1=rtmp, op=Max
                            )
                        nc.vector.tensor_tensor(
                            out=tst_b,
                            in0=ps_t8,
                            in1=kstrip_t[:, h * 8 : (h + 1) * 8, :],
                            op=Mult,
                        )
                        nc.sync.dma_start(
                            out=fscr[
                                h * 8 : (h + 1) * 8, :, mt * P : (mt + 1) * P
                            ].rearrange("t p m -> p t m"),
                            in_=tst_b,
                        )
                    # inva = QTOP/rowmax -> broadcast along free dim via rank-1
                    inva_f = sml.tile([P, 8], F32, tag="invaf")
                    nc.vector.reciprocal(inva_f, rmax_h)
                    inva_b = sml.tile([P, 8], BF16, tag="invab")
                    nc.vector.tensor_scalar(
                        out=inva_b, in0=inva_f, scalar1=QTOP, scalar2=None, op0=Mult
                    )
                    ps_iv = psum.tile([8, P], BF16, tag="ps")
                    nc.tensor.transpose(ps_iv, inva_b, ident_t)
                    ivs = sml.tile([8, P], BF16, tag="ivs")
                    nc.vector.tensor_copy(ivs, ps_iv)
                    ivd = dsc.tile([8, P], BF16, tag="ivd")
                    nc.sync.dma_start(out=ivd[:, :], in_=ivs)
                    iva_row = rows.tile([1, 1024], BF16, tag="ivarow")
                    nc.sync.dma_start(
                        out=iva_row, in_=ivd[:, :].rearrange("t p -> (t p)")
                    )
                    abc_h = rows.tile([P, 1024], BF16, tag="abch")
                    for c2 in range(2):
                        ps_bc = psum.tile([P, 512], F32, tag="ps")
                        nc.tensor.matmul(
                            ps_bc,
                            onesrow_t,
                            iva_row[0:1, c2 * 512 : (c2 + 1) * 512],
                            start=True,
                            stop=True,
                        )
                        nc.vector.tensor_copy(
                            abc_h[:, c2 * 512 : (c2 + 1) * 512], ps_bc
                        )
                    # quantize: Ga = G * inva -> fp8 hi + fp8 lo residual
                    gh8 = q8p.tile([P, MT, 1024], FP8, tag="gh8")
                    gl8 = q8p.tile([P, MT, 1024], FP8, tag="gl8")
                    for mt in range(MT):
                        tq = tmpq.tile([P, 1024], BF16, tag="tq")
                        nc.vector.tensor_tensor(
                            out=tq, in0=g_t[:, mt, :], in1=abc_h, op=Mult
                        )
                        if _NO_ACT8:
                            nc.vector.tensor_copy(gh8[:, mt, :], tq)
                        else:
                            nc.scalar.activation(gh8[:, mt, :], tq, Copy)
                        nc.vector.tensor_tensor(
                            out=gl8[:, mt, :], in0=tq, in1=gh8[:, mt, :], op=Subtract
                        )
                    # s_row over this n-half from quantized weights
                    sraw = sml.tile([P, 8], F32, tag="sraw")
                    for c2 in range(2):
                        ps_row = psum.tile([P, 512], F32, tag="ps")
                        nmm = MT
                        k = 0
                        for mtp in range(MT // 2):
                            for gq in (gh8, gl8):
                                dr_matmul(
                                    ps_row,
                                    ones8_t,
                                    gq[
                                        :,
                                        2 * mtp : 2 * mtp + 2,
                                        c2 * 512 : (c2 + 1) * 512,
                                    ],
                                    start=(k == 0),
                                    stop=(k == nmm - 1),
                                )
                                k += 1
                        srow_row = scr.tile([1, 512], F32, tag="scrow")
                        nc.vector.tensor_copy(srow_row, ps_row[0:1, :])
                        for j in range(4):
                            ps_sr = psum.tile([P, 1], F32, tag="ps")
                            nc.tensor.matmul(
                                ps_sr,
                                srow_row[0:1, j * P : (j + 1) * P],
                                one11_t,
                                start=True,
                                stop=True,
                            )
                            nc.vector.tensor_copy(
                                sraw[:, c2 * 4 + j : c2 * 4 + j + 1], ps_sr
                            )
                    nc.vector.reciprocal(srow_rec[:, h * 8 : (h + 1) * 8], sraw)
                    # attn_a for this n-half: 3-product fp8 DoubleRow
                    for dch in range(2):
                        psu = [
                            psum.tile([P, 512], F32, tag="ps", name=f"psu{_j}")
                            for _j in range(8)
                        ]
                        for mtp in range(MT // 2):
                            vh_t = vst.tile([P, 2, 512], FP8, tag="valh")
                            vl_t = vst.tile([P, 2, 512], FP8, tag="vall")
                            for v_t, src in ((vh_t, x2h8), (vl_t, x2l8)):
                                nc.sync.dma_start(
                                    out=v_t,
                                    in_=src.ap()[
                                        b : b + 1,
                                        2 * mtp : 2 * mtp + 2,
                                        :,
                                        dch * 512 : (dch + 1) * 512,
                                    ].rearrange("o t p d -> p (o t) d"),
                                )
                            for j in range(8):
                                wsl = slice(j * P, (j + 1) * P)
                                ghs = gh8[:, 2 * mtp : 2 * mtp + 2, wsl]
                                gls = gl8[:, 2 * mtp : 2 * mtp + 2, wsl]
                                dr_matmul(
                                    psu[j], ghs, vh_t,
                                    start=(mtp == 0), stop=False,
                                )
                                dr_matmul(
                                    psu[j], ghs, vl_t,
                                    start=False, stop=False,
                                )
                                dr_matmul(
                                    psu[j], gls, vh_t,
                                    start=False, stop=(mtp == MT // 2 - 1),
                                )
                        for j in range(8):
                            nt = h * 8 + j
                            st = stg.tile([P, 512], F32, tag="stage")
                            nc.vector.tensor_scalar(
                                out=st,
                                in0=psu[j],
                                scalar1=srow_rec[:, nt : nt + 1],
                                scalar2=None,
                                op0=Mult,
                            )
                            nc.sync.dma_start(
                                out=outa.ap()[
                                    b : b + 1,
                                    nt : nt + 1,
                                    :,
                                    dch * 512 : (dch + 1) * 512,
                                ].rearrange("o t p d -> p (o t d)"),
                                in_=st,
                            )

                # ---- invb broadcast row for phase B ----
                cmax = sml.tile([P, MT], F32, tag="cmax")
                nc.vector.tensor_tensor(
                    out=cmax, in0=cmax_h[:, 0, :], in1=cmax_h[:, 1, :], op=Max
                )
                nc.vector.tensor_scalar(
                    out=cmax, in0=cmax, scalar1=1e-30, scalar2=None, op0=Max
                )
                invb_f = sml.tile([P, MT], F32, tag="invbf")
                nc.vector.reciprocal(invb_f, cmax)
                invb_b = sml.tile([P, MT], BF16, tag="invbb")
                nc.vector.tensor_scalar(
                    out=invb_b, in0=invb_f, scalar1=QTOP, scalar2=None, op0=Mult
                )
                ps_ivb = psum.tile([MT, P], BF16, tag="ps")
                nc.tensor.transpose(ps_ivb, invb_b, ident_t)
                ivbs = sml.tile([MT, P], BF16, tag="ivbs")
                nc.vector.tensor_copy(ivbs, ps_ivb)
                ivbd = dsc.tile([MT, P], BF16, tag="ivbd")
                nc.sync.dma_start(out=ivbd[:, :], in_=ivbs)
                ivb_row = rows.tile([1, Mm], BF16, tag="ivbrow")
                nc.sync.dma_start(
                    out=ivb_row, in_=ivbd[:, :].rearrange("t p -> (t p)")
                )

                # ---- PHASE B (per m-quarter): attn_b from quantized F strips
                for q in range(4):
                    mq = q * 512
                    bbcq = tmpq.tile([P, 512], BF16, tag="bbcq")
                    ps_bc = psum.tile([P, 512], F32, tag="ps")
                    nc.tensor.matmul(
                        ps_bc,
                        onesrow_t,
                        ivb_row[0:1, mq : mq + 512],
                        start=True,
                        stop=True,
                    )
                    nc.vector.tensor_copy(bbcq, ps_bc)
                    fh8 = f8b.tile([P, NT, 512], FP8, tag="fh8")
                    fl8 = f8b.tile([P, NT, 512], FP8, tag="fl8")
                    for ntp in range(NT // 2):
                        ft_s = fts.tile([P, 2, 512], BF16, tag="fts")
                        nc.sync.dma_start(
                            out=ft_s,
                            in_=fscr[
                                2 * ntp : 2 * ntp + 2, :, mq : mq + 512
                            ].rearrange("t p m -> p t m"),
                        )
                        for k in range(2):
                            nt = 2 * ntp + k
                            tq = tmpq.tile([P, 512], BF16, tag="tq2")
                            nc.vector.tensor_tensor(
                                out=tq, in0=ft_s[:, k, :], in1=bbcq, op=Mult
                            )
                            if _NO_ACT8:
                                nc.vector.tensor_copy(fh8[:, nt, :], tq)
                            else:
                                nc.scalar.activation(fh8[:, nt, :], tq, Copy)
                            nc.vector.tensor_tensor(
                                out=fl8[:, nt, :], in0=tq, in1=fh8[:, nt, :],
                                op=Subtract,
                            )
                    # s_col from quantized weights (keep rows only)
                    ps_sc = psum.tile([P, 512], F32, tag="ps", name="pssc")
                    nmm = NT
                    k = 0
                    for ntp in range(NT // 2):
                        for fq in (fh8, fl8):
                            dr_matmul(
                                ps_sc,
                                keep8_t[:, 2 * ntp : 2 * ntp + 2, :],
                                fq[:, 2 * ntp : 2 * ntp + 2, :],
                                start=(k == 0),
                                stop=(k == nmm - 1),
                            )
                            k += 1
                    scol_row = scr.tile([1, 512], F32, tag="scrow")
                    nc.vector.tensor_copy(scol_row, ps_sc[0:1, :])
                    scraw = sml.tile([P, 4], F32, tag="scraw")
                    for j in range(4):
                        ps_c = psum.tile([P, 1], F32, tag="ps")
                        nc.tensor.matmul(
                            ps_c,
                            scol_row[0:1, j * P : (j + 1) * P],
                            one11_t,
                            start=True,
                            stop=True,
                        )
                        nc.vector.tensor_copy(scraw[:, j : j + 1], ps_c)
                    # += m2*2048 for x2-masked columns (uniform-mean blend denom)
                    nc.vector.tensor_tensor(
                        out=scraw,
                        in0=scraw,
                        in1=m2k_t[:, q * 4 : (q + 1) * 4],
                        op=Add,
                    )
                    nc.vector.reciprocal(scol_rec[:, q * 4 : (q + 1) * 4], scraw)
                    for dch in range(2):
                        psv = [
                            psum.tile([P, 512], F32, tag="ps", name=f"psv{_j}")
                            for _j in range(4)
                        ]
                        for ntp in range(NT // 2):
                            uh_t = vst.tile([P, 2, 512], FP8, tag="valh")
                            ul_t = vst.tile([P, 2, 512], FP8, tag="vall")
                            for v_t, src in ((uh_t, x1h8), (ul_t, x1l8)):
                                nc.sync.dma_start(
                                    out=v_t,
                                    in_=src.ap()[
                                        b : b + 1,
                                        2 * ntp : 2 * ntp + 2,
                                        :,
                                        dch * 512 : (dch + 1) * 512,
                                    ].rearrange("o t p d -> p (o t) d"),
                                )
                            for j in range(4):
                                wsl = slice(j * P, (j + 1) * P)
                                fhs = fh8[:, 2 * ntp : 2 * ntp + 2, wsl]
                                fls = fl8[:, 2 * ntp : 2 * ntp + 2, wsl]
                                dr_matmul(
                                    psv[j], fhs, uh_t,
                                    start=(ntp == 0), stop=False,
                                )
                                dr_matmul(
                                    psv[j], fhs, ul_t,
                                    start=False, stop=False,
                                )
                                dr_matmul(
                                    psv[j], fls, uh_t,
                                    start=False,
                                    stop=(ntp == NT // 2 - 1),
                                )
                            if ntp == 0:
                                # mean blend for x2-masked cols, mid-group so a
                                # tracked matmul closes the accumulation
                                for j in range(4):
                                    mt = q * 4 + j
                                    nc.tensor.matmul(
                                        psv[j],
                                        m2i_t[0:1, mt * P : (mt + 1) * P],
                                        blr_t[0:1, dch * 512 : (dch + 1) * 512],
                                        start=False,
                                        stop=False,
                                        skip_group_check=True,
                                    )
                        for j in range(4):
                            mt = q * 4 + j
                            st = stg.tile([P, 512], F32, tag="stage")
                            nc.vector.tensor_scalar(
                                out=st,
                                in0=psv[j],
                                scalar1=scol_rec[:, mt : mt + 1],
                                scalar2=None,
                                op0=Mult,
                            )
                            nc.sync.dma_start(
                                out=outb.ap()[
                                    b : b + 1,
                                    mt : mt + 1,
                                    :,
                                    dch * 512 : (dch + 1) * 512,
                                ].rearrange("o t p d -> p (o t d)"),
                                in_=st,
                            )


_NC_CACHE = None


def _get_nc():
    global _NC_CACHE
    if _NC_CACHE is None:
        nc = bacc.Bacc("TRN2", target_bir_lowering=False, debug=False)
        _emit(nc)
        nc.compile()
        _NC_CACHE = nc
    return _NC_CACHE


def _hi_lo8(x):
    hi = np.asarray(x, FP8_NP)
    lo = np.asarray(x - hi.astype(np.float32), FP8_NP)
    return hi, lo


def _prep_in_maps(x1, x1_mask, x2, x2_mask, W1, b1, W2, b2):
    f32 = np.float32
    x1 = np.ascontiguousarray(x1, f32)
    x2 = np.ascontiguousarray(x2, f32)
    W1 = np.ascontiguousarray(W1, f32)
    W2 = np.ascontiguousarray(W2, f32)
    b1 = np.asarray(b1, f32)
    b2 = np.asarray(b2, f32)
    m1 = np.asarray(x1_mask, bool)
    m2 = np.asarray(x2_mask, bool)

    w1t = np.ascontiguousarray(W1.T).reshape(DT, P, D)
    w2t = np.ascontiguousarray(W2.T).reshape(DT, P, D)
    b1c = np.ascontiguousarray(b1.reshape(ET, P).T)
    b2c = np.ascontiguousarray(b2.reshape(ET, P).T)
    ident = np.eye(P, dtype=BF16_NP)
    ones8 = np.ones((P, 2, P), FP8_NP)
    onesrow = np.ones((1, P), BF16_NP)
    one11 = np.ones((1, 1), f32)
    c2048 = np.full((1, 1), 2048.0, BF16_NP)

    in_maps = []
    for c in range(NCORES):
        sl = slice(c * BPC, (c + 1) * BPC)
        x1c, x2c = x1[sl], x2[sl]
        m1c, m2c = m1[sl], m2[sl]
        x1tc = np.ascontiguousarray(x1c.transpose(0, 2, 1)).reshape(BPC, DT, P, Nn)
        x2tc = np.ascontiguousarray(x2c.transpose(0, 2, 1)).reshape(BPC, DT, P, Mm)
        x1z = np.where(m1c[:, :, None], 0.0, x1c).astype(f32)
        x1h, x1l = _hi_lo8(x1z)
        x2h, x2l = _hi_lo8(x2c)
        x1hc = np.ascontiguousarray(x1h).reshape(BPC, NT, P, D)
        x1lc = np.ascontiguousarray(x1l).reshape(BPC, NT, P, D)
        x2hc = np.ascontiguousarray(x2h).reshape(BPC, MT, P, D)
        x2lc = np.ascontiguousarray(x2l).reshape(BPC, MT, P, D)
        x2mb = np.where(m2c, np.float64(NEG), 0.0) - C_SHIFT
        x2mbc = np.ascontiguousarray(
            x2mb.astype(f32).reshape(BPC, MT, P).transpose(0, 2, 1)
        )
        m2kc = np.ascontiguousarray(
            (m2c.astype(f32) * 2048.0).reshape(BPC, MT, P).transpose(0, 2, 1)
        )
        keep1 = (~m1c).astype(FP8_NP)
        keep8c = np.ascontiguousarray(keep1.reshape(BPC, NT, P).transpose(0, 2, 1))
        keep8sc = np.ascontiguousarray(
            np.broadcast_to(keep8c.reshape(BPC, P, NT, 1), (BPC, P, NT, P))
        )
        keeprow = (~m1c).astype(BF16_NP).reshape(BPC, 1, Nn)
        kstrip = np.ascontiguousarray(
            np.broadcast_to(keep8c.reshape(BPC, P, NT, 1), (BPC, P, NT, P))
        )
        m2i = m2c.astype(BF16_NP).reshape(BPC, 1, Mm)
        blrow = x1c.sum(axis=1, dtype=np.float64).astype(BF16_NP).reshape(BPC, 1, D)
        in_maps.append(
            {
                "x1t": x1tc,
                "x2t": x2tc,
                "w1t": w1t,
                "w2t": w2t,
                "b1c": b1c,
                "b2c": b2c,
                "x1h8": x1hc,
                "x1l8": x1lc,
                "x2h8": x2hc,
                "x2l8": x2lc,
                "x2mbc": x2mbc,
                "m2k": m2kc,
                "keep8s": keep8sc,
                "keeprow": keeprow,
                "keepstrip": kstrip,
                "m2i": m2i,
                "blr": blrow,
                "ident": ident,
                "ones8": ones8,
                "onesrow": onesrow,
                "one11": one11,
                "c2048": c2048,
            }
        )
    return in_maps


def kernel(x1, x1_mask, x2, x2_mask, W1, b1, W2, b2, _trace=False):
    nc = _get_nc()
    in_maps = _prep_in_maps(x1, x1_mask, x2, x2_mask, W1, b1, W2, b2)
    res = run_bass_kernel_spmd(nc, in_maps, core_ids=list(range(NCORES)), trace=_trace)
    attn_a = np.empty((B, Nn, D), np.float32)
    attn_b = np.empty((B, Mm, D), np.float32)
    for c in range(NCORES):
        sl = slice(c * BPC, (c + 1) * BPC)
        attn_a[sl] = res.results[c]["outa"].reshape(BPC, Nn, D)
        attn_b[sl] = res.results[c]["outb"].reshape(BPC, Mm, D)
    if _trace:
        kernel._last_exec_time_ns = res.exec_time_ns
        kernel._last_results = res
    return attn_a, attn_b
